# revision 94
# baseline (speedup 1.0000x reference)
"""Trainium2 Bass kernel for nn_BasicTransformerBlock (cross-attention block).

Reference computation (per batch b of 16):
  q = x[b] @ Wq                        [4096, 512]
  k/v    = ctx_txt[b] @ Wk/Wv          [77, 512]
  k/v_ip = ctx_img[b] @ Wk_ip/Wv_ip    [16, 512]
  per head h (8 heads, d=64):
    sim = q_h @ k_h.T * 0.125, softmax over keys (txt / img separately)
    out_h = ts * softmax(sim_txt) @ v_txt + is * softmax(sim_img) @ v_img
  out = merge_heads(out) @ Wo + bo     [4096, 320]

Sharding: data-parallel over batch, 2 batches per core on 8 cores.

Kernel structure (per core), v10 (~164us vs 254us baseline):
  - Fused epilogue via associativity: VW_h = V_h @ Wo_h [keys, 320] per head
    (text/img output scales folded into V), so each output chunk is one PSUM
    accumulation  out = sum_h probsT_h.T @ VW_h  with no intermediate
    attention-output tensor or separate out-projection. bo is folded into
    VW_0's text rows (softmax text probs sum to 1), so no bias matmul.
  - Q projection runs in fp8e4m3 with DoubleRow perf mode (2 matmuls per
    m-tile over a 2x(2x128) packed contraction, 0.5 cycles/row): x and Wq
    are quantized on the host (x padded to 512 qd so both DR pairs are
    full). Measured end-to-end rel err 1.48e-2 vs the 2e-2 gate (fixed
    seed => deterministic).
  - Keys packed contiguously: txt at 0:77, img at 77:93, zero pad 93:96.
    QK/exp span 96 columns; the 128-col probs blocks (required by the DMA
    xbar transpose granularity) carry stale cols 96:128 that are never read
    downstream (PV contracts partitions 0:96 only).
  - Software-pipelined unit loop (16 units = 2 batches x 8 groups of 512
    tokens) with a LAG-3 epilogue: iteration i runs Qproj_i + QK/softmax_i
    interleaved with the PV+store of unit i-3, so the PE never waits on the
    exp->reduce->normalize->transpose chain (~6us after its unit's QK).
  - K/V/VW projections are split into pieces and scheduled into the tails of
    the first iterations so the PE is never serially blocked on weight DMA;
    the big weight loads are split in half so the early x/Wq loads are not
    stuck behind 3us transfers on the shared DMA engines.
  - x and context are packed on the host into per-partition-contiguous
    layouts (>=512B DMA descriptors = full DMA rate).
  - PSUM: qproj ring 4 banks, sim ring 2, pv ring 2 (tuned by sweep).
  - Output stores ride the SP(sync) ring: a DMA holds its issuing ring's
    sequencer while WAITING on its input, so stores on the ACT ring blocked
    exp dispatch and paced the whole pipeline (-23us when moved).
  - Engine balance per unit: ACT = exp + 2 qproj copies + 4 out copies +
    x loads (ring); DVE = reductions + reciprocal + img normalize + 2 qproj
    copies; Pool(gpsimd) = txt normalize (gpsimd cannot touch PSUM on HW);
    SP ring = xbar transpose + output stores; SWDGE = weights.
    All five engines land at 60-72% busy (ACT 118us / PE 117 / DVE 113 /
    DMA 112 / Pool 105 of 164us total).
"""
import sys

if "/opt/trn_rl_repo" not in sys.path:
    sys.path.insert(0, "/opt/trn_rl_repo")

import ml_dtypes
import numpy as np

import concourse.bacc as bacc
import concourse.mybir as mybir
import concourse.tile as tile
from concourse.bass_utils import run_bass_kernel_spmd

F32 = mybir.dt.float32
BF16 = mybir.dt.bfloat16
F8 = mybir.dt.float8e4
AF = mybir.ActivationFunctionType
ALU = mybir.AluOpType
X_AX = mybir.AxisListType.X

N_CORES = 8
B = 16
BPC = B // N_CORES          # batches per core
N = 4096                    # tokens
QD = 320                    # query dim
CD = 1024                   # context dim
H = 8                       # heads
D = 64                      # head dim
ID = H * D                  # 512
TXT = 77                    # text keys
IMG = 16                    # image keys
KEYS = TXT + IMG            # 93 packed keys
KSPAN = 96                  # padded key span (pad 93:96 zeroed)
NG = 8                      # groups of 512 tokens per batch
SCALE = 0.125               # 1/sqrt(64)

_NC_CACHE = None


def _build_nc():
    nc = bacc.Bacc("TRN2", target_bir_lowering=False, debug=False)

    # x pre-packed on host: x[b, p, g, 384*c + 128*k + m]
    #   = x_orig[b, 512*g + 128*c + m, 128*k + p]   (zero pad for k=2, p>=64)
    x = nc.dram_tensor("x", [BPC, 128, NG, 2048], F8,
                       kind="ExternalInput").ap()
    # context pre-packed on host: ctx[b, p, 96*k + key] = ctx_orig[b, key, 128*k+p]
    # with txt keys at 0:77, img keys at 77:93, zero pad 93:96
    ctx = nc.dram_tensor("context", [BPC, 128, 768], BF16,
                         kind="ExternalInput").ap()
    # Wq padded to [512, ID] (rows 320:512 zero), fp8 for DoubleRow matmul
    Wq = nc.dram_tensor("Wq", [512, ID], F8, kind="ExternalInput").ap()
    Wk = nc.dram_tensor("Wk", [CD, ID], BF16, kind="ExternalInput").ap()
    Wv = nc.dram_tensor("Wv", [CD, ID], BF16, kind="ExternalInput").ap()
    Wk_ip = nc.dram_tensor("Wk_ip", [CD, ID], BF16, kind="ExternalInput").ap()
    Wv_ip = nc.dram_tensor("Wv_ip", [CD, ID], BF16, kind="ExternalInput").ap()
    Wo = nc.dram_tensor("Wo", [ID, QD], BF16, kind="ExternalInput").ap()
    bo = nc.dram_tensor("bo", [QD], BF16, kind="ExternalInput").ap()
    tscale = nc.dram_tensor("text_scale", [1], F32, kind="ExternalInput").ap()
    iscale = nc.dram_tensor("img_scale", [1], F32, kind="ExternalInput").ap()
    out = nc.dram_tensor("out", [BPC, N, QD], F32, kind="ExternalOutput").ap()

    with tile.TileContext(nc) as tc:
        with tc.tile_pool(name="wpool", bufs=1) as wpool, \
             tc.tile_pool(name="kvpool", bufs=2) as kvpool, \
             tc.tile_pool(name="upool", bufs=1) as upool, \
             tc.tile_pool(name="pp", bufs=2, space="PSUM") as pp:

            units = [(b, g) for b in range(BPC) for g in range(NG)]
            NU = len(units)

            # ---------------- early loads (issue order matters) ------------
            xt_ring = []

            def load_x(idx):
                b, g = units[idx]
                t = upool.tile([128, 2, 2, 4, 128], F8, name=f"xt{idx}",
                               tag="xt", bufs=3)
                nc.scalar.dma_start(
                    out=t.rearrange("p r k c m -> p (r k c m)"),
                    in_=x[b, :, g, :])
                return t

            wq = wpool.tile([128, 4, ID], F8)
            nc.scalar.dma_start(
                out=wq[:],
                in_=Wq.rearrange("(k p) m -> p k m", p=128))

            xt_ring.append(load_x(0))
            xt_ring.append(load_x(1))

            ctxt = []
            for b in range(BPC):
                t = kvpool.tile([128, 8, 96], BF16, name=f"ctxt{b}",
                                tag="ctx", bufs=2)
                nc.scalar.dma_start(out=t.rearrange("p k c -> p (k c)"),
                                    in_=ctx[b])
                ctxt.append(t)

            bo_row = wpool.tile([1, QD], BF16)
            nc.scalar.dma_start(out=bo_row[:], in_=bo[None, :])
            ts_sb = wpool.tile([1, 1], F32)
            nc.scalar.dma_start(out=ts_sb[:], in_=tscale[:, None])
            is_sb = wpool.tile([1, 1], F32)
            nc.scalar.dma_start(out=is_sb[:], in_=iscale[:, None])

            # weights on SWDGE (bf16 from host)
            def load_w(dram_ap, kt_count, mdim, name):
                wbf = wpool.tile([128, kt_count, mdim], BF16, name=f"w_{name}")
                nc.gpsimd.dma_start(
                    out=wbf[:],
                    in_=dram_ap.rearrange("(k p) m -> p k m", p=128))
                return wbf

            wk = wpool.tile([128, 8, ID], BF16, name="w_wk")
            nc.gpsimd.dma_start(
                out=wk[:, 0:4, :],
                in_=Wk[0:512, :].rearrange("(k p) m -> p k m", p=128))
            nc.gpsimd.dma_start(
                out=wk[:, 4:8, :],
                in_=Wk[512:1024, :].rearrange("(k p) m -> p k m", p=128))
            wkip = wpool.tile([128, 8, ID], BF16, name="w_wkip")
            nc.gpsimd.dma_start(
                out=wkip[:, 0:4, :],
                in_=Wk_ip[0:512, :].rearrange("(k p) m -> p k m", p=128))
            nc.gpsimd.dma_start(
                out=wkip[:, 4:8, :],
                in_=Wk_ip[512:1024, :].rearrange("(k p) m -> p k m", p=128))
            wv = wpool.tile([128, 8, ID], BF16, name="w_wv")
            nc.gpsimd.dma_start(
                out=wv[:, 0:4, :],
                in_=Wv[0:512, :].rearrange("(k p) m -> p k m", p=128))
            nc.gpsimd.dma_start(
                out=wv[:, 4:8, :],
                in_=Wv[512:1024, :].rearrange("(k p) m -> p k m", p=128))
            wvip = wpool.tile([128, 8, ID], BF16, name="w_wvip")
            nc.gpsimd.dma_start(
                out=wvip[:, 0:4, :],
                in_=Wv_ip[0:512, :].rearrange("(k p) m -> p k m", p=128))
            nc.gpsimd.dma_start(
                out=wvip[:, 4:8, :],
                in_=Wv_ip[512:1024, :].rearrange("(k p) m -> p k m", p=128))
            wo = load_w(Wo, 4, QD, "wo")

            ts_col = wpool.tile([128, 1], F32)
            nc.gpsimd.partition_broadcast(ts_col[:], ts_sb[:])
            is_col = wpool.tile([128, 1], F32)
            nc.gpsimd.partition_broadcast(is_col[:], is_sb[:])
            bo_bcast = wpool.tile([128, QD], BF16)
            nc.gpsimd.partition_broadcast(bo_bcast[:], bo_row[:])

            # ---------------- per-unit Q projection ------------------------
            def qproj(xt, idx):
                qt = upool.tile([128, 4, 512], BF16, name=f"qt{idx}",
                                tag="qt", bufs=2)
                for m in range(4):
                    psum_q = pp.tile([128, 512], F32, tag="qproj", bufs=4,
                                     name="psum_q")
                    for pr in range(2):
                        nc.tensor.matmul(
                            psum_q[:],
                            wq[:, 2 * pr:2 * pr + 2, 128 * m:128 * (m + 1)],
                            xt[:, pr, :, :, :],
                            start=(pr == 0), stop=(pr == 1),
                            perf_mode=mybir.MatmulPerfMode.DoubleRow)
                    if m == 0 or m == 1:
                        nc.vector.tensor_copy(qt[:, m, :], psum_q[:])
                    else:
                        # (gpsimd cannot read PSUM on HW; keep these on ACT)
                        nc.scalar.activation(qt[:, m, :], psum_q[:], AF.Copy)
                return qt

            # Qproj for units 0 and 1 up front (PE work while weights load)
            qt_ring = [qproj(xt_ring[0], 0), qproj(xt_ring[1], 1)]

            # ------------- per-batch K^T / V^T / VW projection pieces -------
            kts = {}
            vts = {}
            vws = {}

            def kv_kt(b):
                ct = ctxt[b]
                psum_kt = pp.tile([128, 4, KSPAN], F32, tag="sim", bufs=2,
                                  name="psum_kt")
                for m in range(4):
                    for k in range(8):
                        nc.tensor.matmul(
                            psum_kt[:, m, 0:TXT],
                            wk[:, k, 128 * m:128 * (m + 1)],
                            ct[:, k, 0:TXT],
                            start=(k == 0), stop=(k == 7))
                for m in range(4):
                    for k in range(8):
                        nc.tensor.matmul(
                            psum_kt[:, m, TXT:KEYS],
                            wkip[:, k, 128 * m:128 * (m + 1)],
                            ct[:, k, TXT:KEYS],
                            start=(k == 0), stop=(k == 7))
                kt = wpool.tile([128, 4, KSPAN], BF16, name=f"kt{b}")
                nc.gpsimd.memset(kt[:], 0.0)
                nc.vector.tensor_copy(kt[:, :, 0:KEYS], psum_kt[:, :, 0:KEYS])
                kts[b] = kt

            def kv_vt(b):
                ct = ctxt[b]
                psum_vt = pp.tile([128, 4, KSPAN], F32, tag="sim", bufs=2,
                                  name="psum_vt")
                for m in range(4):
                    for k in range(8):
                        nc.tensor.matmul(
                            psum_vt[:, m, 0:TXT],
                            wv[:, k, 128 * m:128 * (m + 1)],
                            ct[:, k, 0:TXT],
                            start=(k == 0), stop=(k == 7))
                for m in range(4):
                    for k in range(8):
                        nc.tensor.matmul(
                            psum_vt[:, m, TXT:KEYS],
                            wvip[:, k, 128 * m:128 * (m + 1)],
                            ct[:, k, TXT:KEYS],
                            start=(k == 0), stop=(k == 7))
                vt = kvpool.tile([128, 4, KSPAN], BF16, name=f"vt{b}",
                                 tag="vt", bufs=2)
                nc.gpsimd.memset(vt[:], 0.0)
                nc.vector.tensor_scalar_mul(vt[:, :, 0:TXT],
                                            psum_vt[:, :, 0:TXT],
                                            ts_col[:, 0:1])
                nc.vector.tensor_scalar_mul(vt[:, :, TXT:KEYS],
                                            psum_vt[:, :, TXT:KEYS],
                                            is_col[:, 0:1])
                vts[b] = vt

            def kv_vw(b):
                # VW_h = V_h @ Wo_h  [keys, 320] per head; zero pad rows come
                # from vt's zero pad columns.
                vt = vts[b]
                vw = wpool.tile([128, 8, QD], BF16, name=f"vw{b}")
                for h in range(H):
                    hp, hh = h // 2, h % 2
                    psum_vw = pp.tile([128, QD], F32, tag="pv", bufs=2,
                                      name="psum_vw")
                    nc.tensor.matmul(
                        psum_vw[0:KSPAN, :],
                        vt[64 * hh:64 * (hh + 1), hp, :],
                        wo[64 * hh:64 * (hh + 1), hp, :],
                        start=True, stop=True)
                    if h % 2 == 0:
                        nc.vector.tensor_copy(vw[0:KSPAN, h, :],
                                              psum_vw[0:KSPAN, :])
                    else:
                        nc.scalar.activation(vw[0:KSPAN, h, :],
                                             psum_vw[0:KSPAN, :], AF.Copy)
                # fold bo into head 0's text rows: sum_k softmax_txt = 1
                nc.gpsimd.tensor_add(
                    vw[0:TXT, 0, :], vw[0:TXT, 0, :], bo_bcast[0:TXT, :])
                vws[b] = vw

            kv_kt(0)

            # pieces scheduled into iteration tails: by the end of iter 1 we
            # need vw0 (PV_0 runs in iter 2) and kt1 only by iter 8.
            tail_work = {
                1: [lambda: kv_vt(0), lambda: kv_vw(0), lambda: kv_kt(1)],
                2: [lambda: kv_vt(1)],
                3: [lambda: kv_vw(1)],
            }

            # ---------------- software-pipelined unit loop ------------------
            LAG = 3
            pending = []  # [(probsT, b, g, out4), ...] lag-LAG queue

            def pv_chunks(prevstate, js):
                probsT, b, g, out4 = prevstate
                vw = vws[b]
                for j in js:
                    psum_o = pp.tile([128, QD], F32, tag="pv", bufs=2,
                                     name="psum_o")
                    for h in range(H):
                        nc.tensor.matmul(
                            psum_o[:],
                            probsT[0:KSPAN, 4 * h + j, :],
                            vw[0:KSPAN, h, :],
                            start=(h == 0), stop=(h == 7))
                    nc.scalar.activation(out4[:, j, :], psum_o[:], AF.Copy)

            def store_prev(prevstate):
                _, b, g, out4 = prevstate
                nc.sync.dma_start(
                    out=out[b, 512 * g:512 * (g + 1), :]
                        .rearrange("(j p) d -> p j d", p=128),
                    in_=out4[:])

            for idx in range(NU):
                b, g = units[idx]
                if idx + 2 < NU:
                    xt_ring.append(load_x(idx + 2))
                # guaranteed-ready PV work first: keeps the PE fed while the
                # qproj copies of the previous unit drain (psum WAR)
                if len(pending) == LAG:
                    pv_chunks(pending[0], (0, 1))
                if idx >= 2:
                    qt_ring.append(qproj(xt_ring[idx], idx))
                qt = qt_ring[idx]
                kt = kts[b]

                probs = upool.tile([128, 8, 4, 128], BF16, name=f"probs{idx}",
                                   tag="probs", bufs=3)
                dsum = upool.tile([128, 8, 2, 4], F32, name=f"dsum{idx}",
                                  tag="dsum", bufs=2)
                rsum = upool.tile([128, 8, 2, 4], F32, name=f"rsum{idx}",
                                  tag="rsum", bufs=2)

                def head_group(hps):
                    for hp in hps:
                        for hh in range(2):
                            h = 2 * hp + hh
                            psum_s = pp.tile([128, 4, KSPAN], F32, tag="sim",
                                             bufs=2, name="psum_s")
                            for c4 in range(4):
                                nc.tensor.matmul(
                                    psum_s[:, c4, :],
                                    qt[64 * hh:64 * (hh + 1), hp,
                                       128 * c4:128 * (c4 + 1)],
                                    kt[64 * hh:64 * (hh + 1), hp, :],
                                    start=True, stop=True)
                            nc.scalar.activation(
                                probs[:, h, :, 0:KSPAN],
                                psum_s[:], AF.Exp, scale=SCALE)
                            nc.vector.reduce_sum(
                                out=dsum[:, h, 0, :],
                                in_=probs[:, h, :, 0:TXT], axis=X_AX)
                            nc.vector.reduce_sum(
                                out=dsum[:, h, 1, :],
                                in_=probs[:, h, :, TXT:KEYS], axis=X_AX)
                        h0 = 2 * hp
                        nc.vector.reciprocal(rsum[:, h0:h0 + 2, :, :],
                                             dsum[:, h0:h0 + 2, :, :])
                        for hh in range(2):
                            h = 2 * hp + hh
                            nc.gpsimd.tensor_mul(
                                probs[:, h, :, 0:TXT],
                                probs[:, h, :, 0:TXT],
                                rsum[:, h, 0, :][:, :, None]
                                    .broadcast_to([128, 4, TXT]))
                            nc.vector.tensor_mul(
                                probs[:, h, :, TXT:KEYS],
                                probs[:, h, :, TXT:KEYS],
                                rsum[:, h, 1, :][:, :, None]
                                    .broadcast_to([128, 4, IMG]))

                head_group((0, 1))
                if len(pending) == LAG:
                    pv_chunks(pending[0], (2, 3))
                head_group((2, 3))
                if len(pending) == LAG:
                    done = pending.pop(0)
                    store_prev(done)

                probsT = upool.tile([128, 32, 128], BF16, name=f"probsT{idx}",
                                    tag="probsT", bufs=LAG + 1)
                nc.sync.dma_start(
                    out=probsT[:],
                    in_=probs.rearrange("p h c k -> p (h c k)"),
                    transpose=True)
                out4 = upool.tile([128, 4, QD], F32, name=f"out4{idx}",
                                  tag="out4", bufs=LAG + 1)
                pending.append((probsT, b, g, out4))

                for work in tail_work.get(idx, ()):
                    work()

            for pi, prevstate in enumerate(pending):
                pv_chunks(prevstate, (0, 1))
                if pi == len(pending) - 1:
                    _, db, dg, dout4 = prevstate
                    nc.sync.dma_start(
                        out=out[db, 512 * dg:512 * dg + 256, :]
                            .rearrange("(j p) d -> p j d", p=128),
                        in_=dout4[:, 0:2, :])
                    pv_chunks(prevstate, (2, 3))
                    nc.sync.dma_start(
                        out=out[db, 512 * dg + 256:512 * (dg + 1), :]
                            .rearrange("(j p) d -> p j d", p=128),
                        in_=dout4[:, 2:4, :])
                else:
                    pv_chunks(prevstate, (2, 3))
                    store_prev(prevstate)

    nc.compile()
    return nc


def _get_nc():
    global _NC_CACHE
    if _NC_CACHE is None:
        _NC_CACHE = _build_nc()
    return _NC_CACHE


def _pack_x(x):
    # [B, N, QD] f32 -> [B, 128(p), NG(g), 2048] fp8e4m3, value at
    # [b, p, g, 1024r+512k+128c+m] = x[b, 512g+128c+m, 128(2r+k)+p]
    # (zero pad for qd >= 320)
    xf8 = np.asarray(x, np.float32).astype(ml_dtypes.float8_e4m3fn)
    xf8 = xf8.reshape(B, NG, 4, 128, QD)                # b, g, c, m, qd
    xp = np.zeros((B, NG, 4, 128, 512), ml_dtypes.float8_e4m3fn)
    xp[:, :, :, :, 0:QD] = xf8
    xp = xp.reshape(B, NG, 4, 128, 2, 2, 128)           # b, g, c, m, r, k, p
    xp = xp.transpose(0, 6, 1, 4, 5, 2, 3)              # b, p, g, r, k, c, m
    return np.ascontiguousarray(xp.reshape(B, 128, NG, 2048))


def _pack_ctx(context):
    # [B, 93, CD] f32 -> [B, 128(p), 768] bf16 with txt keys at 0:77,
    # img keys at 77:93, zero pad 93:96 per 96-key block
    cbf = np.asarray(context, np.float32).astype(ml_dtypes.bfloat16)
    cbf = cbf.reshape(B, 93, 8, 128).transpose(0, 3, 2, 1)  # b, p, k, key93
    cp = np.zeros((B, 128, 8, 96), ml_dtypes.bfloat16)
    cp[:, :, :, 0:KEYS] = cbf
    return np.ascontiguousarray(cp.reshape(B, 128, 768))


def _pack_wq(Wq):
    # [QD, ID] f32 -> [512, ID] fp8e4m3 with rows 320:512 zeroed
    wq8 = np.zeros((512, ID), ml_dtypes.float8_e4m3fn)
    wq8[0:QD] = np.asarray(Wq, np.float32).astype(ml_dtypes.float8_e4m3fn)
    return np.ascontiguousarray(wq8)


def kernel(x, context, Wq, Wk, Wv, Wk_ip, Wv_ip, Wo, bo, text_scale, img_scale):
    x = _pack_x(x)
    context = _pack_ctx(context)
    bf = lambda a: np.ascontiguousarray(
        np.asarray(a, np.float32).astype(ml_dtypes.bfloat16))
    shared = {
        "Wq": _pack_wq(Wq),
        "Wk": bf(Wk), "Wv": bf(Wv), "Wk_ip": bf(Wk_ip),
        "Wv_ip": bf(Wv_ip), "Wo": bf(Wo), "bo": bf(bo),
        "text_scale": np.asarray(text_scale, np.float32),
        "img_scale": np.asarray(img_scale, np.float32),
    }
    nc = _get_nc()
    in_maps = []
    for c in range(N_CORES):
        m = dict(shared)
        m["x"] = x[BPC * c:BPC * (c + 1)]
        m["context"] = context[BPC * c:BPC * (c + 1)]
        in_maps.append(m)
    res = run_bass_kernel_spmd(nc, in_maps, core_ids=list(range(N_CORES)))
    return np.concatenate([res.results[c]["out"] for c in range(N_CORES)], axis=0)
